# revision 51
# baseline (speedup 1.0000x reference)
"""GraphSAGE 2-layer mini-batch kernel for 8 Trainium2 NeuronCores.

Strategy: data-parallel over the batch (128 targets per core), x replicated.
The dominant cost is gathering ~36.6K random 512B rows of x per core.

Big gathers (nb1_self, nb1_nb: 35200 rows in 11 "chains" of 128 groups x 25)
use the Q7 dma_gather custom instruction (int16 indices, bucket-sorted into
16 buckets of 32768 rows).  Each bucket's gather is split into ~4
sub-instructions and buckets are issued round-robin across the 4 SWDGE
queues: queue q's descriptors are generated by Q7 core pair q, and the Pool
engine pipelines up to 4 instructions, so all four pairs generate
descriptors concurrently instead of serializing on one pair.

Aggregation: bucketing permutes rows, so mean-aggregation uses
data-dependent selection matrices built on-device (is_equal of per-row
local group ids against an iota).  Each 128-row tile gets ONE exact-width
window (the sorted rows of a tile span only ~82-220 consecutive group ids,
far less than the 11*128 group space), built per-bucket in a single DVE op.
Gathered tiles are cast to bf16 on the Scalar engine and the selection
matmuls run in bf16 (fast stationary loads + 2x column rate), accumulating
f32 in PSUM [feature, group].  The 1/25 and 1/10 mean scalings are folded
into host-prescaled copies of W1/W2.

nodes + nb2 (1408 rows) are gathered in exact order with indirect DMA
(issued after the big-gather dispatch; they only feed the SAGE layers) and
transposed on the PE.

Both SAGE layers run in transposed [feature, row] layout: W is the
stationary matmul operand, L2 norms use a ones-vector matmul for the
cross-partition reduction, a rank-1 matmul to broadcast the norm back
across partitions, and an elementwise divide.
"""
import sys

sys.path.insert(0, "/opt/trn_rl_repo")

import numpy as np

P = 128
D = 128
B = 1024
S1 = 25
S2 = 10
N_NODES = 500000
NCORES = 8
B_LOC = B // NCORES          # 128 targets per core
NCHAIN = 11                  # nb1_self + 10 nb1_nb chains
BUCKET_BITS = 15
BUCKET = 1 << BUCKET_BITS    # 32768 rows per bucket (int16 addressable)
NBUK = (N_NODES + BUCKET - 1) // BUCKET  # 16
NQ = 4                       # SWDGE queues
SUB_PARTS = 4                # sub-gathers per bucket (pipeline across queues)
PSUM_BANK = 512              # f32 cols per PSUM bank


def _prep_indices(nodes, nb2, nb1_self, nb1_nb):
    """Bucket-sort the big-gather indices per core; build device-side arrays
    and the (core-independent) per-tile window metadata."""
    per_core = []
    for c in range(NCORES):
        sl = slice(c * B_LOC, (c + 1) * B_LOC)
        n1s = nb1_self[sl]              # [128, 25]
        n1n = nb1_nb[sl]                # [128, 10, 25]
        idx_chains = [n1s.reshape(-1).astype(np.int64)]
        grp_chains = [np.repeat(np.arange(B_LOC, dtype=np.int64), S1)]
        for j in range(S2):
            idx_chains.append(n1n[:, j, :].reshape(-1).astype(np.int64))
            grp_chains.append((j + 1) * B_LOC
                              + np.repeat(np.arange(B_LOC, dtype=np.int64), S1))
        all_idx = np.concatenate(idx_chains)   # [35200]
        all_grp = np.concatenate(grp_chains)
        bkt = all_idx >> BUCKET_BITS
        order = np.argsort(bkt, kind="stable")
        sidx, sgrp, sbkt = all_idx[order], all_grp[order], bkt[order]
        locs, grps = [], []
        for b in range(NBUK):
            m = sbkt == b
            locs.append((sidx[m] - (b << BUCKET_BITS)).astype(np.int64))
            grps.append(sgrp[m])
        per_core.append((locs, grps))

    # consistent per-bucket tile counts across cores (SPMD: one program)
    Cb = [max((len(per_core[c][0][b]) + P - 1) // P for c in range(NCORES))
          for b in range(NBUK)]
    Cb = [max(cb, 1) for cb in Cb]
    T_total = sum(Cb)
    S_total = T_total * P

    idx16_cores, grp_glob = [], []
    for c in range(NCORES):
        locs, grps = per_core[c]
        lidx = np.zeros(S_total, np.int64)
        lgrp = np.full(S_total, -1000.0, np.float64)
        off = 0
        for b in range(NBUK):
            n = len(locs[b])
            lidx[off:off + n] = locs[b]
            lgrp[off:off + n] = grps[b]
            off += Cb[b] * P
        wrapped_cols = []
        off = 0
        for b in range(NBUK):
            nb_pad = Cb[b] * P
            w = lidx[off:off + nb_pad].reshape(-1, 16).T.astype(np.int16)
            wrapped_cols.append(np.tile(w, (8, 1)))
            off += nb_pad
        idx16_cores.append(np.hstack(wrapped_cols))        # [128, S_total//16]
        grp_glob.append(lgrp.reshape(T_total, P).T)        # [128, T_total]

    # first tile per bucket that may contain padding on some core (memset
    # target: gather skips trailing -1 rows, slots must be zero not garbage)
    pad_tile0 = []
    for b in range(NBUK):
        mn = min(len(per_core[c][0][b]) for c in range(NCORES))
        pad_tile0.append(mn // P)

    # per-tile exact windows, unioned across cores so metadata is SPMD-safe
    base_t = np.zeros(T_total, np.int64)
    width_t = np.ones(T_total, np.int64)
    for t in range(T_total):
        mn, mx = None, None
        for c in range(NCORES):
            g = grp_glob[c][:, t]
            v = g[g >= 0]
            if v.size:
                lo, hi = int(v.min()), int(v.max())
                mn = lo if mn is None else min(mn, lo)
                mx = hi if mx is None else max(mx, hi)
        if mn is None:
            base_t[t], width_t[t] = 0, 2
        else:
            base_t[t] = mn & ~1               # even base: bf16 4B alignment
            width_t[t] = mx - base_t[t] + 1

    # uniform window width per bucket (one batched DVE sel build per bucket)
    tile_base = np.cumsum([0] + Cb)
    Wb = []
    for b in range(NBUK):
        w = int(max(width_t[tile_base[b]:tile_base[b + 1]]))
        Wb.append((w + 7) & ~7)
    W_max = max(Wb)

    # matmul pieces per tile: window clipped to [0, NCHAIN*P), split at PSUM
    # bank boundaries.  (psum_col0, psum_col1) global agg columns.
    pieces_t = []
    for t in range(T_total):
        lo = int(base_t[t])
        hi = min(lo + int(width_t[t]), NCHAIN * P)
        ps = []
        col = lo
        while col < hi:
            nxt = min(hi, (col // PSUM_BANK + 1) * PSUM_BANK)
            ps.append((col, nxt))
            col = nxt
        pieces_t.append(ps)

    # per-core grp data: tile-local values
    grp_tile_cores = []
    for c in range(NCORES):
        gl = grp_glob[c].astype(np.float64)
        gt = np.where(gl >= 0, gl - base_t[None, :], -1000.0)
        grp_tile_cores.append(gt.astype(np.float32))

    return dict(Cb=Cb, T_total=T_total, tile_base=tile_base,
                base_t=base_t, Wb=Wb, W_max=W_max, pieces_t=pieces_t,
                pad_tile0=pad_tile0,
                idx16_cores=idx16_cores, grp_tile_cores=grp_tile_cores)


def _build_program(meta, trace_sim=False, debug=False):
    import concourse.bacc as bacc_mod
    import concourse.bass as bass
    import concourse.tile as tile
    from concourse import mybir

    f32 = mybir.dt.float32
    f32r = mybir.dt.float32r
    bf16 = mybir.dt.bfloat16
    Cb = meta["Cb"]
    T_total = meta["T_total"]
    tile_base = meta["tile_base"]
    base_t = meta["base_t"]
    Wb = meta["Wb"]
    W_max = meta["W_max"]
    pieces_t = meta["pieces_t"]

    nc = bacc_mod.Bacc(num_swdge_queues=NQ)

    x_d = nc.declare_dram_parameter("x", [N_NODES, D], bf16, isOutput=False)
    w1a_d = nc.declare_dram_parameter("w1a", [D, D], f32, isOutput=False)
    w1b_d = nc.declare_dram_parameter("w1b", [D, D], f32, isOutput=False)
    w2a_d = nc.declare_dram_parameter("w2a", [D, D], f32, isOutput=False)
    w2b_d = nc.declare_dram_parameter("w2b", [D, D], f32, isOutput=False)
    b1_d = nc.declare_dram_parameter("b1v", [D, 1], f32, isOutput=False)
    b2_d = nc.declare_dram_parameter("b2v", [D, 1], f32, isOutput=False)
    ident_d = nc.declare_dram_parameter("ident", [P, P], f32, isOutput=False)
    ones_d = nc.declare_dram_parameter("onesm", [P, P], f32, isOutput=False)
    iota_d = nc.declare_dram_parameter("iota", [P, max(Cb) * W_max], bf16,
                                       isOutput=False)
    idx16_d = nc.declare_dram_parameter("idx16", [P, T_total * 8],
                                        mybir.dt.int16, isOutput=False)
    grp_d = nc.declare_dram_parameter("grp", [P, T_total], bf16, isOutput=False)
    idx32_d = nc.declare_dram_parameter("idx32", [P, NCHAIN],
                                        mybir.dt.int32, isOutput=False)
    zt_d = nc.declare_dram_parameter("zt", [D, B_LOC], f32, isOutput=True)
    dbg_d = None
    if debug:
        dbg_d = nc.declare_dram_parameter("dbg", [D, (NCHAIN + 1) * P], f32,
                                          isOutput=True)

    with tile.TileContext(nc, trace_sim=trace_sim) as tc:
        with (
            tc.tile_pool(name="consts", bufs=1) as consts,
            tc.tile_pool(name="acts", bufs=1) as acts,
            tc.tile_pool(name="gbuf", bufs=10) as gpool,
            tc.tile_pool(name="lg", bufs=22) as lgpool,
            tc.tile_pool(name="selp", bufs=5) as selpool,
            tc.tile_pool(name="scratch", bufs=16) as scratch,
        ):
          with (
            tc.tile_pool(name="pagg", bufs=1, space="PSUM") as pagg,
            tc.tile_pool(name="ptr", bufs=2, space="PSUM") as ptr,
          ):
              # ---- gather-critical loads first ------------------------------
              idx16a = consts.tile([P, T_total * 8], mybir.dt.int16,
                                   tag="idx16a", name="idx16a")
              nc.sync.dma_start(out=idx16a[:], in_=idx16_d[:])
              idx32 = consts.tile([P, NCHAIN], mybir.dt.int32, tag="idx32")
              nc.sync.dma_start(out=idx32[:], in_=idx32_d[:])

              # ---- dispatch big bucketed gathers ----------------------------
              # bucket b -> queue b%4 (desc-gen on Q7 pair b%4); groups of 4
              # adjacent buckets rotate the issue so all four Q7 pairs
              # generate descriptors concurrently.
              gtiles = {}

              def emit_gather_part(b, c0, c1, q):
                  cb = Cb[b]
                  if b not in gtiles:
                      gtiles[b] = gpool.tile([P, cb * P], bf16, tag="gb",
                                             name=f"g{b}")
                  g = gtiles[b]
                  lo = b * BUCKET
                  hi = min(lo + BUCKET, N_NODES)
                  n = c1 - c0
                  g3 = g[:, c0 * D:c1 * D].rearrange("p (c e) -> p c e", c=n)
                  i0, i1 = (tile_base[b] + c0) * 8, (tile_base[b] + c1) * 8
                  nc.gpsimd.dma_gather(
                      out_ap=g3,
                      in_ap=x_d[lo:hi, :],
                      idxs_ap=idx16a[:, i0:i1],
                      num_idxs=n * P,
                      num_idxs_reg=n * P,
                      elem_size=D,
                      single_packet=True,
                      queue_num=q,
                  )

              sub_bounds = []
              for b in range(NBUK):
                  cuts = [round(i * Cb[b] / SUB_PARTS)
                          for i in range(SUB_PARTS + 1)]
                  sub_bounds.append([(cuts[i], cuts[i + 1])
                                     for i in range(SUB_PARTS)
                                     if cuts[i + 1] > cuts[i]])

              lts = []
              for g0 in range(0, NBUK, NQ):
                  for s in range(SUB_PARTS):
                      for b in range(g0, min(g0 + NQ, NBUK)):
                          if s < len(sub_bounds[b]):
                              c0, c1 = sub_bounds[b][s]
                              emit_gather_part(b, c0, c1, b % NQ)
                      # one exact-order indirect gather per wave: they run on
                      # Q7 pair 0 / ring 0 (mainline SWDGE) concurrently with
                      # the bucketed gathers on pairs 1-3, so the SAGE self
                      # terms are ready long before the tail.
                      if len(lts) < NCHAIN:
                          j = len(lts)
                          lt = lgpool.tile([P, D], bf16, tag="lt",
                                           name=f"lt{j}")
                          nc.gpsimd.indirect_dma_start(
                              out=lt[:], out_offset=None, in_=x_d[:],
                              in_offset=bass.IndirectOffsetOnAxis(
                                  ap=idx32[:, j:j + 1], axis=0),
                          )
                          lts.append(lt)

              # ---- remaining const loads ------------------------------------
              grpc = consts.tile([P, T_total], bf16, tag="grpc")
              iota = consts.tile([P, max(Cb) * W_max], bf16, tag="iota")
              nc.sync.dma_start(out=grpc[:], in_=grp_d[:])
              nc.sync.dma_start(out=iota[:], in_=iota_d[:])
              w1a = consts.tile([D, D], f32r, tag="w1a")
              w1b = consts.tile([D, D], f32r, tag="w1b")
              w2a = consts.tile([D, D], f32r, tag="w2a")
              w2b = consts.tile([D, D], f32r, tag="w2b")
              b1t = consts.tile([D, 1], f32, tag="b1t")
              b2t = consts.tile([D, 1], f32, tag="b2t")
              ident = consts.tile([P, P], f32r, tag="ident")
              ones = consts.tile([P, P], f32r, tag="ones")
              eps = consts.tile([P, 1], f32, tag="eps")
              nc.vector.memset(eps[:], 1e-30)
              for dst, srcd in ((w1a, w1a_d), (w1b, w1b_d), (w2a, w2a_d),
                                (w2b, w2b_d)):
                  nc.sync.dma_start(out=dst[:], in_=srcd[:].bitcast(f32r))
              for dst, srcd in ((b1t, b1_d), (b2t, b2_d)):
                  nc.sync.dma_start(out=dst[:], in_=srcd[:])
              nc.sync.dma_start(out=ident[:], in_=ident_d[:].bitcast(f32r))
              nc.sync.dma_start(out=ones[:], in_=ones_d[:].bitcast(f32r))

              # ---- transpose the exact-order rows (SAGE self terms) ---------
              # chain 0 = x[nodes]; chain 1+j = x[nb2[:, j]]
              self_sb = []
              for j in range(NCHAIN):
                  ltf = lgpool.tile([P, D], f32r, tag="ltf", name=f"ltf{j}")
                  nc.scalar.copy(out=ltf[:], in_=lts[j][:])
                  ps = ptr.tile([P, P], f32, tag="pt", name=f"pt{j}")
                  nc.tensor.transpose(out=ps[:].bitcast(f32r), in_=ltf[:],
                                      identity=ident[:])
                  st = acts.tile([D, B_LOC], f32r, tag=f"selfT{j}",
                                 name=f"selfT{j}")
                  nc.scalar.copy(out=st[:], in_=ps[:])
                  self_sb.append(st)

              # ---- PSUM accumulation bookkeeping ----------------------------
              agg_ps = [pagg.tile([P, 4 * P], f32, tag=f"agg{k}", name=f"agg{k}")
                        for k in range(3)]

              # bucket consumption order matches expected gather completion
              BORDER = [0, 1, 2, 3, 4, 5, 6, 7, 8, 9, 10, 11, 15, 12, 13, 14]
              piece_list = []    # (t, col0, col1) in emission order
              for b in BORDER:
                  for tl in range(Cb[b]):
                      t = tile_base[b] + tl
                      for (col0, col1) in pieces_t[t]:
                          piece_list.append((t, col0, col1))
              first_pi, last_pi = {}, {}
              for i, (t, col0, col1) in enumerate(piece_list):
                  bank = col0 // PSUM_BANK
                  if bank not in first_pi:
                      first_pi[bank] = i
                  last_pi[bank] = i

              # ---- per-bucket: sel build + agg matmuls ----------------------
              pi = 0
              for b in BORDER:
                  cb = Cb[b]
                  tb = tile_base[b]
                  wb = Wb[b]
                  g16 = gtiles[b]
                  sel = selpool.tile([P, cb * wb], bf16, tag="sel",
                                     name=f"sel{b}")
                  nc.vector.tensor_tensor(
                      out=sel[:].rearrange("p (c w) -> p c w", c=cb),
                      in0=grpc[:, tb:tb + cb].broadcast_to([P, cb, wb]),
                      in1=iota[:, :cb * W_max].rearrange(
                          "p (c w) -> p c w", c=cb)[:, :, :wb],
                      op=mybir.AluOpType.is_equal,
                  )
                  for tl in range(cb):
                      t = tb + tl
                      gt16 = g16[:, tl * D:(tl + 1) * D]
                      for (col0, col1) in pieces_t[t]:
                          bank = col0 // PSUM_BANK
                          off = col0 - bank * PSUM_BANK
                          l0 = col0 - int(base_t[t])
                          l1 = col1 - int(base_t[t])
                          nc.tensor.matmul(
                              out=agg_ps[bank][:, off:off + (col1 - col0)],
                              lhsT=gt16,
                              rhs=sel[:, tl * wb + l0:tl * wb + l1],
                              start=(first_pi[bank] == pi),
                              stop=(last_pi[bank] == pi),
                              skip_group_check=True,
                          )
                          pi += 1

              # ---- copy aggregated sums PSUM -> SBUF ------------------------
              def agg_slice(ch):
                  return agg_ps[ch // 4][:, (ch % 4) * P:(ch % 4 + 1) * P]

              agg_sb = []
              for ch in range(NCHAIN):
                  a = acts.tile([D, B_LOC], f32r, tag=f"aggT{ch}",
                                name=f"aggT{ch}")
                  nc.scalar.copy(out=a[:], in_=agg_slice(ch))
                  agg_sb.append(a)

              if debug:
                  for ch in range(NCHAIN):
                      nc.sync.dma_start(
                          out=dbg_d[:, ch * P:(ch + 1) * P].bitcast(f32r),
                          in_=agg_sb[ch][:])
                  nc.sync.dma_start(
                      out=dbg_d[:, NCHAIN * P:(NCHAIN + 1) * P].bitcast(f32r),
                      in_=self_sb[0][:])

          # ---- SAGE layer in transposed layout (agg PSUM banks now free) --
          with tc.tile_pool(name="psage", bufs=8, space="PSUM") as psage:
            h1n_all = acts.tile([P, S2 * P], f32r, tag="h1n_all")

            def sage_group(specs):
                """Stage-major emission of several independent SAGE heads so
                the engines pipeline across them."""
                phs, hs, h2s, psss, nvs, pbcs, nrs = [], [], [], [], [], [], []
                for i, (rs, ra, wa, wb_, bt, tagn, hn) in enumerate(specs):
                    ph = psage.tile([P, P], f32, tag="ps", name=f"ph_{tagn}")
                    nc.tensor.matmul(out=ph[:], lhsT=wa[:], rhs=rs,
                                     start=True, stop=False,
                                     skip_group_check=True)
                    nc.tensor.matmul(out=ph[:], lhsT=wb_[:], rhs=ra,
                                     start=False, stop=True,
                                     skip_group_check=True)
                    phs.append(ph)
                for i, (rs, ra, wa, wb_, bt, tagn, hn) in enumerate(specs):
                    h = scratch.tile([P, P], f32, tag="h", name=f"h_{tagn}")
                    nc.vector.tensor_scalar(out=h[:], in0=phs[i][:],
                                            scalar1=bt[:, :1], scalar2=0.0,
                                            op0=mybir.AluOpType.add,
                                            op1=mybir.AluOpType.max)
                    hs.append(h)
                for i, (rs, ra, wa, wb_, bt, tagn, hn) in enumerate(specs):
                    h2 = scratch.tile([P, P], f32r, tag="h2", name=f"h2_{tagn}")
                    nc.scalar.square(out=h2[:], in_=hs[i][:])
                    h2s.append(h2)
                for i, (rs, ra, wa, wb_, bt, tagn, hn) in enumerate(specs):
                    pss = psage.tile([P, P], f32, tag="ps", name=f"pss_{tagn}")
                    nc.tensor.matmul(out=pss[:1, :], lhsT=ones[:, :1],
                                     rhs=h2s[i][:], start=True, stop=True,
                                     skip_group_check=True)
                    psss.append(pss)
                for i, (rs, ra, wa, wb_, bt, tagn, hn) in enumerate(specs):
                    nv = scratch.tile([P, P], f32r, tag="nv", name=f"nv_{tagn}")
                    nc.scalar.activation(
                        out=nv[:1, :], in_=psss[i][:1, :],
                        func=mybir.ActivationFunctionType.Sqrt,
                        bias=eps[:1, :1])
                    nvs.append(nv)
                for i, (rs, ra, wa, wb_, bt, tagn, hn) in enumerate(specs):
                    pbc = psage.tile([P, P], f32, tag="ps", name=f"pbc_{tagn}")
                    nc.tensor.matmul(out=pbc[:], lhsT=ones[:1, :],
                                     rhs=nvs[i][:1, :], start=True, stop=True,
                                     skip_group_check=True)
                    pbcs.append(pbc)
                for i, (rs, ra, wa, wb_, bt, tagn, hn) in enumerate(specs):
                    nr = scratch.tile([P, P], f32, tag="nr", name=f"nr_{tagn}")
                    nc.vector.reciprocal_approx_fast(out=nr[:], in_=pbcs[i][:])
                    nrs.append(nr)
                outs = []
                for i, (rs, ra, wa, wb_, bt, tagn, hn) in enumerate(specs):
                    if hn is None:
                        hn = acts.tile([D, B_LOC], f32r, tag=tagn,
                                       name=tagn)[:]
                    nc.vector.tensor_tensor(out=hn, in0=hs[i][:], in1=nrs[i][:],
                                            op=mybir.AluOpType.mult)
                    outs.append(hn)
                return outs

            h1n_slice = lambda j: h1n_all[:, j * P:(j + 1) * P]
            specs = [(self_sb[0][:], agg_sb[0][:], w1a, w1b, b1t, "h1t", None)]
            specs += [(self_sb[1 + j][:], agg_sb[1 + j][:], w1a, w1b, b1t,
                       f"h1n{j}", h1n_slice(j)) for j in range(S2)]
            res0 = sage_group(specs)
            h1t = res0[0]

            a3 = acts.tile([D, B_LOC], f32r, tag="a3")
            with nc.allow_low_precision("f32r is 4-byte fp32 bits"):
                nc.vector.reduce_sum(
                    out=a3[:],
                    in_=h1n_all[:].rearrange("p (j r) -> p r j", j=S2),
                    axis=mybir.AxisListType.X,
                )

            zt = sage_group([(h1t, a3[:], w2a, w2b, b2t, "zt", None)])[0]
            nc.sync.dma_start(out=zt_d[:].bitcast(f32r), in_=zt)

    nc.finalize()
    return nc


def kernel(x, W1, b1, W2, b2, nodes, nb2, nb1_self, nb1_nb,
           _trace=False, _core_ids=None, _debug=False):
    x = np.ascontiguousarray(np.asarray(x, dtype=np.float32))
    W1 = np.asarray(W1, dtype=np.float32)
    W2 = np.asarray(W2, dtype=np.float32)
    b1 = np.asarray(b1, dtype=np.float32)
    b2 = np.asarray(b2, dtype=np.float32)
    nodes = np.asarray(nodes)
    nb2 = np.asarray(nb2)
    nb1_self = np.asarray(nb1_self)
    nb1_nb = np.asarray(nb1_nb)

    meta = _prep_indices(nodes, nb2, nb1_self, nb1_nb)
    nc = _build_program(meta, debug=_debug)

    try:
        import ml_dtypes
        bf16_np = ml_dtypes.bfloat16
    except ImportError:
        bf16_np = None

    def to_bf16(a):
        if bf16_np is not None:
            return np.ascontiguousarray(a.astype(bf16_np))
        b = a.astype(np.float32).view(np.uint32)
        b = ((b + 0x8000 + ((b >> 16) & 1)) >> 16).astype(np.uint16)
        return np.ascontiguousarray(b)

    # host-prescaled weights: the 1/25 and 1/10 means fold into W*b
    w1a = np.ascontiguousarray(W1[:D])
    w1b = np.ascontiguousarray(W1[D:] / S1)
    w2a = np.ascontiguousarray(W2[:D])
    w2b = np.ascontiguousarray(W2[D:] / S2)
    ident = np.eye(P, dtype=np.float32)
    x16 = to_bf16(x)
    W_max = meta["W_max"]
    Cmax = max(meta["Cb"])
    iota1 = np.tile(np.arange(W_max, dtype=np.float32), Cmax)
    iota = np.broadcast_to(iota1, (P, Cmax * W_max))
    iota16 = to_bf16(iota)

    in_maps = []
    for c in range(NCORES):
        sl = slice(c * B_LOC, (c + 1) * B_LOC)
        idx32 = np.empty((P, NCHAIN), np.int32)
        idx32[:, 0] = nodes[sl]
        idx32[:, 1:] = nb2[sl]
        in_maps.append({
            "x": x16,
            "w1a": w1a, "w1b": w1b, "w2a": w2a, "w2b": w2b,
            "b1v": b1.reshape(D, 1), "b2v": b2.reshape(D, 1),
            "ident": ident, "iota": iota16,
            "onesm": np.ones((P, P), np.float32),
            "idx16": meta["idx16_cores"][c],
            "grp": to_bf16(meta["grp_tile_cores"][c]),
            "idx32": idx32,
        })

    from concourse.bass_utils import run_bass_kernel_spmd

    core_ids = _core_ids if _core_ids is not None else list(range(NCORES))
    res = run_bass_kernel_spmd(nc, in_maps[:len(core_ids)], core_ids=core_ids,
                               trace=_trace)
    z = np.concatenate([res.results[c]["zt"].T for c in range(len(core_ids))],
                       axis=0)
    kernel.last_exec_time_ns = res.exec_time_ns
    kernel.last_results = res
    return z


# revision 53
# speedup vs baseline: 1.0239x; 1.0239x over previous
"""GraphSAGE 2-layer mini-batch kernel for 8 Trainium2 NeuronCores.

Strategy: data-parallel over the batch (128 targets per core), x replicated.
The dominant cost is gathering ~36.6K random 512B rows of x per core.

Big gathers (nb1_self, nb1_nb: 35200 rows in 11 "chains" of 128 groups x 25)
use the Q7 dma_gather custom instruction (int16 indices, bucket-sorted into
16 buckets of 32768 rows).  Each bucket's gather is split into ~4
sub-instructions and buckets are issued round-robin across the 4 SWDGE
queues: queue q's descriptors are generated by Q7 core pair q, and the Pool
engine pipelines up to 4 instructions, so all four pairs generate
descriptors concurrently instead of serializing on one pair.

Aggregation: bucketing permutes rows, so mean-aggregation uses
data-dependent selection matrices built on-device (is_equal of per-row
local group ids against an iota).  Each 128-row tile gets ONE exact-width
window (the sorted rows of a tile span only ~82-220 consecutive group ids,
far less than the 11*128 group space), built per-bucket in a single DVE op.
Gathered tiles are cast to bf16 on the Scalar engine and the selection
matmuls run in bf16 (fast stationary loads + 2x column rate), accumulating
f32 in PSUM [feature, group].  The 1/25 and 1/10 mean scalings are folded
into host-prescaled copies of W1/W2.

nodes + nb2 (1408 rows) are gathered in exact order with indirect DMA
(issued after the big-gather dispatch; they only feed the SAGE layers) and
transposed on the PE.

Both SAGE layers run in transposed [feature, row] layout: W is the
stationary matmul operand, L2 norms use a ones-vector matmul for the
cross-partition reduction, a rank-1 matmul to broadcast the norm back
across partitions, and an elementwise divide.
"""
import sys

sys.path.insert(0, "/opt/trn_rl_repo")

import numpy as np

P = 128
D = 128
B = 1024
S1 = 25
S2 = 10
N_NODES = 500000
NCORES = 8
B_LOC = B // NCORES          # 128 targets per core
NCHAIN = 11                  # nb1_self + 10 nb1_nb chains
BUCKET_BITS = 15
BUCKET = 1 << BUCKET_BITS    # 32768 rows per bucket (int16 addressable)
NBUK = (N_NODES + BUCKET - 1) // BUCKET  # 16
NQ = 4                       # SWDGE queues
SUB_PARTS = 5                # sub-gathers per bucket (pipeline across queues)
PSUM_BANK = 512              # f32 cols per PSUM bank


def _prep_indices(nodes, nb2, nb1_self, nb1_nb):
    """Bucket-sort the big-gather indices per core; build device-side arrays
    and the (core-independent) per-tile window metadata."""
    per_core = []
    for c in range(NCORES):
        sl = slice(c * B_LOC, (c + 1) * B_LOC)
        n1s = nb1_self[sl]              # [128, 25]
        n1n = nb1_nb[sl]                # [128, 10, 25]
        idx_chains = [n1s.reshape(-1).astype(np.int64)]
        grp_chains = [np.repeat(np.arange(B_LOC, dtype=np.int64), S1)]
        for j in range(S2):
            idx_chains.append(n1n[:, j, :].reshape(-1).astype(np.int64))
            grp_chains.append((j + 1) * B_LOC
                              + np.repeat(np.arange(B_LOC, dtype=np.int64), S1))
        all_idx = np.concatenate(idx_chains)   # [35200]
        all_grp = np.concatenate(grp_chains)
        bkt = all_idx >> BUCKET_BITS
        order = np.argsort(bkt, kind="stable")
        sidx, sgrp, sbkt = all_idx[order], all_grp[order], bkt[order]
        locs, grps = [], []
        for b in range(NBUK):
            m = sbkt == b
            locs.append((sidx[m] - (b << BUCKET_BITS)).astype(np.int64))
            grps.append(sgrp[m])
        per_core.append((locs, grps))

    # consistent per-bucket tile counts across cores (SPMD: one program)
    Cb = [max((len(per_core[c][0][b]) + P - 1) // P for c in range(NCORES))
          for b in range(NBUK)]
    Cb = [max(cb, 1) for cb in Cb]
    T_total = sum(Cb)
    S_total = T_total * P

    idx16_cores, grp_glob = [], []
    for c in range(NCORES):
        locs, grps = per_core[c]
        lidx = np.zeros(S_total, np.int64)
        lgrp = np.full(S_total, -1000.0, np.float64)
        off = 0
        for b in range(NBUK):
            n = len(locs[b])
            lidx[off:off + n] = locs[b]
            lgrp[off:off + n] = grps[b]
            off += Cb[b] * P
        wrapped_cols = []
        off = 0
        for b in range(NBUK):
            nb_pad = Cb[b] * P
            w = lidx[off:off + nb_pad].reshape(-1, 16).T.astype(np.int16)
            wrapped_cols.append(np.tile(w, (8, 1)))
            off += nb_pad
        idx16_cores.append(np.hstack(wrapped_cols))        # [128, S_total//16]
        grp_glob.append(lgrp.reshape(T_total, P).T)        # [128, T_total]

    # first tile per bucket that may contain padding on some core (memset
    # target: gather skips trailing -1 rows, slots must be zero not garbage)
    pad_tile0 = []
    for b in range(NBUK):
        mn = min(len(per_core[c][0][b]) for c in range(NCORES))
        pad_tile0.append(mn // P)

    # per-tile exact windows, unioned across cores so metadata is SPMD-safe
    base_t = np.zeros(T_total, np.int64)
    width_t = np.ones(T_total, np.int64)
    for t in range(T_total):
        mn, mx = None, None
        for c in range(NCORES):
            g = grp_glob[c][:, t]
            v = g[g >= 0]
            if v.size:
                lo, hi = int(v.min()), int(v.max())
                mn = lo if mn is None else min(mn, lo)
                mx = hi if mx is None else max(mx, hi)
        if mn is None:
            base_t[t], width_t[t] = 0, 2
        else:
            base_t[t] = mn & ~1               # even base: bf16 4B alignment
            width_t[t] = mx - base_t[t] + 1

    # uniform window width per bucket (one batched DVE sel build per bucket)
    tile_base = np.cumsum([0] + Cb)
    Wb = []
    for b in range(NBUK):
        w = int(max(width_t[tile_base[b]:tile_base[b + 1]]))
        Wb.append((w + 7) & ~7)
    W_max = max(Wb)

    # matmul pieces per tile: window clipped to [0, NCHAIN*P), split at PSUM
    # bank boundaries.  (psum_col0, psum_col1) global agg columns.
    pieces_t = []
    for t in range(T_total):
        lo = int(base_t[t])
        hi = min(lo + int(width_t[t]), NCHAIN * P)
        ps = []
        col = lo
        while col < hi:
            nxt = min(hi, (col // PSUM_BANK + 1) * PSUM_BANK)
            ps.append((col, nxt))
            col = nxt
        pieces_t.append(ps)

    # per-core grp data: tile-local values
    grp_tile_cores = []
    for c in range(NCORES):
        gl = grp_glob[c].astype(np.float64)
        gt = np.where(gl >= 0, gl - base_t[None, :], -1000.0)
        grp_tile_cores.append(gt.astype(np.float32))

    return dict(Cb=Cb, T_total=T_total, tile_base=tile_base,
                base_t=base_t, Wb=Wb, W_max=W_max, pieces_t=pieces_t,
                pad_tile0=pad_tile0,
                idx16_cores=idx16_cores, grp_tile_cores=grp_tile_cores)


def _build_program(meta, trace_sim=False, debug=False):
    import concourse.bacc as bacc_mod
    import concourse.bass as bass
    import concourse.tile as tile
    from concourse import mybir

    f32 = mybir.dt.float32
    f32r = mybir.dt.float32r
    bf16 = mybir.dt.bfloat16
    Cb = meta["Cb"]
    T_total = meta["T_total"]
    tile_base = meta["tile_base"]
    base_t = meta["base_t"]
    Wb = meta["Wb"]
    W_max = meta["W_max"]
    pieces_t = meta["pieces_t"]

    nc = bacc_mod.Bacc(num_swdge_queues=NQ)

    x_d = nc.declare_dram_parameter("x", [N_NODES, D], bf16, isOutput=False)
    w1a_d = nc.declare_dram_parameter("w1a", [D, D], f32, isOutput=False)
    w1b_d = nc.declare_dram_parameter("w1b", [D, D], f32, isOutput=False)
    w2a_d = nc.declare_dram_parameter("w2a", [D, D], f32, isOutput=False)
    w2b_d = nc.declare_dram_parameter("w2b", [D, D], f32, isOutput=False)
    b1_d = nc.declare_dram_parameter("b1v", [D, 1], f32, isOutput=False)
    b2_d = nc.declare_dram_parameter("b2v", [D, 1], f32, isOutput=False)
    ident_d = nc.declare_dram_parameter("ident", [P, P], f32, isOutput=False)
    ones_d = nc.declare_dram_parameter("onesm", [P, P], f32, isOutput=False)
    iota_d = nc.declare_dram_parameter("iota", [P, max(Cb) * W_max], bf16,
                                       isOutput=False)
    idx16_d = nc.declare_dram_parameter("idx16", [P, T_total * 8],
                                        mybir.dt.int16, isOutput=False)
    grp_d = nc.declare_dram_parameter("grp", [P, T_total], bf16, isOutput=False)
    idx32_d = nc.declare_dram_parameter("idx32", [P, NCHAIN],
                                        mybir.dt.int32, isOutput=False)
    zt_d = nc.declare_dram_parameter("zt", [D, B_LOC], f32, isOutput=True)
    dbg_d = None
    if debug:
        dbg_d = nc.declare_dram_parameter("dbg", [D, (NCHAIN + 1) * P], f32,
                                          isOutput=True)

    with tile.TileContext(nc, trace_sim=trace_sim) as tc:
        with (
            tc.tile_pool(name="consts", bufs=1) as consts,
            tc.tile_pool(name="acts", bufs=1) as acts,
            tc.tile_pool(name="gbuf", bufs=10) as gpool,
            tc.tile_pool(name="lg", bufs=22) as lgpool,
            tc.tile_pool(name="selp", bufs=5) as selpool,
            tc.tile_pool(name="scratch", bufs=16) as scratch,
        ):
          with (
            tc.tile_pool(name="pagg", bufs=1, space="PSUM") as pagg,
            tc.tile_pool(name="ptr", bufs=2, space="PSUM") as ptr,
          ):
              # ---- gather-critical loads first ------------------------------
              idx16a = consts.tile([P, T_total * 8], mybir.dt.int16,
                                   tag="idx16a", name="idx16a")
              nc.sync.dma_start(out=idx16a[:], in_=idx16_d[:])
              idx32 = consts.tile([P, NCHAIN], mybir.dt.int32, tag="idx32")
              nc.sync.dma_start(out=idx32[:], in_=idx32_d[:])

              # ---- dispatch big bucketed gathers ----------------------------
              # bucket b -> queue b%4 (desc-gen on Q7 pair b%4); groups of 4
              # adjacent buckets rotate the issue so all four Q7 pairs
              # generate descriptors concurrently.
              gtiles = {}

              def emit_gather_part(b, c0, c1, q):
                  cb = Cb[b]
                  if b not in gtiles:
                      gtiles[b] = gpool.tile([P, cb * P], bf16, tag="gb",
                                             name=f"g{b}")
                  g = gtiles[b]
                  lo = b * BUCKET
                  hi = min(lo + BUCKET, N_NODES)
                  n = c1 - c0
                  g3 = g[:, c0 * D:c1 * D].rearrange("p (c e) -> p c e", c=n)
                  i0, i1 = (tile_base[b] + c0) * 8, (tile_base[b] + c1) * 8
                  nc.gpsimd.dma_gather(
                      out_ap=g3,
                      in_ap=x_d[lo:hi, :],
                      idxs_ap=idx16a[:, i0:i1],
                      num_idxs=n * P,
                      num_idxs_reg=n * P,
                      elem_size=D,
                      single_packet=True,
                      queue_num=q,
                  )

              sub_bounds = []
              for b in range(NBUK):
                  cuts = [round(i * Cb[b] / SUB_PARTS)
                          for i in range(SUB_PARTS + 1)]
                  sub_bounds.append([(cuts[i], cuts[i + 1])
                                     for i in range(SUB_PARTS)
                                     if cuts[i + 1] > cuts[i]])

              for g0 in range(0, NBUK, NQ):
                  for s in range(SUB_PARTS):
                      for b in range(g0, min(g0 + NQ, NBUK)):
                          if s < len(sub_bounds[b]):
                              c0, c1 = sub_bounds[b][s]
                              emit_gather_part(b, c0, c1, b % NQ)

              # ---- remaining const loads ------------------------------------
              grpc = consts.tile([P, T_total], bf16, tag="grpc")
              iota = consts.tile([P, max(Cb) * W_max], bf16, tag="iota")
              nc.sync.dma_start(out=grpc[:], in_=grp_d[:])
              nc.sync.dma_start(out=iota[:], in_=iota_d[:])
              w1a = consts.tile([D, D], f32r, tag="w1a")
              w1b = consts.tile([D, D], f32r, tag="w1b")
              w2a = consts.tile([D, D], f32r, tag="w2a")
              w2b = consts.tile([D, D], f32r, tag="w2b")
              b1t = consts.tile([D, 1], f32, tag="b1t")
              b2t = consts.tile([D, 1], f32, tag="b2t")
              ident = consts.tile([P, P], f32r, tag="ident")
              ones = consts.tile([P, P], f32r, tag="ones")
              eps = consts.tile([P, 1], f32, tag="eps")
              nc.vector.memset(eps[:], 1e-30)
              for dst, srcd in ((w1a, w1a_d), (w1b, w1b_d), (w2a, w2a_d),
                                (w2b, w2b_d)):
                  nc.sync.dma_start(out=dst[:], in_=srcd[:].bitcast(f32r))
              for dst, srcd in ((b1t, b1_d), (b2t, b2_d)):
                  nc.sync.dma_start(out=dst[:], in_=srcd[:])
              nc.sync.dma_start(out=ident[:], in_=ident_d[:].bitcast(f32r))
              nc.sync.dma_start(out=ones[:], in_=ones_d[:].bitcast(f32r))

              # ---- little exact-order gathers (SAGE self terms) -------------
              # chain 0 = x[nodes]; chain 1+j = x[nb2[:, j]]
              self_sb = []
              for j in range(NCHAIN):
                  lt = lgpool.tile([P, D], bf16, tag="lt", name=f"lt{j}")
                  nc.gpsimd.indirect_dma_start(
                      out=lt[:], out_offset=None, in_=x_d[:],
                      in_offset=bass.IndirectOffsetOnAxis(ap=idx32[:, j:j + 1],
                                                          axis=0),
                  )
                  ltf = lgpool.tile([P, D], f32r, tag="ltf", name=f"ltf{j}")
                  nc.scalar.copy(out=ltf[:], in_=lt[:])
                  ps = ptr.tile([P, P], f32, tag="pt", name=f"pt{j}")
                  nc.tensor.transpose(out=ps[:].bitcast(f32r), in_=ltf[:],
                                      identity=ident[:])
                  st = acts.tile([D, B_LOC], f32r, tag=f"selfT{j}",
                                 name=f"selfT{j}")
                  nc.scalar.copy(out=st[:], in_=ps[:])
                  self_sb.append(st)

              # ---- PSUM accumulation bookkeeping ----------------------------
              agg_ps = [pagg.tile([P, 4 * P], f32, tag=f"agg{k}", name=f"agg{k}")
                        for k in range(3)]

              # bucket consumption order matches expected gather completion
              BORDER = [0, 1, 2, 3, 4, 5, 6, 7, 8, 9, 10, 11, 15, 12, 13, 14]
              piece_list = []    # (t, col0, col1) in emission order
              for b in BORDER:
                  for tl in range(Cb[b]):
                      t = tile_base[b] + tl
                      for (col0, col1) in pieces_t[t]:
                          piece_list.append((t, col0, col1))
              first_pi, last_pi = {}, {}
              for i, (t, col0, col1) in enumerate(piece_list):
                  bank = col0 // PSUM_BANK
                  if bank not in first_pi:
                      first_pi[bank] = i
                  last_pi[bank] = i

              # ---- per-bucket: sel build + agg matmuls ----------------------
              pi = 0
              for b in BORDER:
                  cb = Cb[b]
                  tb = tile_base[b]
                  wb = Wb[b]
                  g16 = gtiles[b]
                  sel = selpool.tile([P, cb * wb], bf16, tag="sel",
                                     name=f"sel{b}")
                  nc.vector.tensor_tensor(
                      out=sel[:].rearrange("p (c w) -> p c w", c=cb),
                      in0=grpc[:, tb:tb + cb].broadcast_to([P, cb, wb]),
                      in1=iota[:, :cb * W_max].rearrange(
                          "p (c w) -> p c w", c=cb)[:, :, :wb],
                      op=mybir.AluOpType.is_equal,
                  )
                  for tl in range(cb):
                      t = tb + tl
                      gt16 = g16[:, tl * D:(tl + 1) * D]
                      for (col0, col1) in pieces_t[t]:
                          bank = col0 // PSUM_BANK
                          off = col0 - bank * PSUM_BANK
                          l0 = col0 - int(base_t[t])
                          l1 = col1 - int(base_t[t])
                          nc.tensor.matmul(
                              out=agg_ps[bank][:, off:off + (col1 - col0)],
                              lhsT=gt16,
                              rhs=sel[:, tl * wb + l0:tl * wb + l1],
                              start=(first_pi[bank] == pi),
                              stop=(last_pi[bank] == pi),
                              skip_group_check=True,
                          )
                          pi += 1

              # ---- copy aggregated sums PSUM -> SBUF ------------------------
              def agg_slice(ch):
                  return agg_ps[ch // 4][:, (ch % 4) * P:(ch % 4 + 1) * P]

              agg_sb = []
              for ch in range(NCHAIN):
                  a = acts.tile([D, B_LOC], f32r, tag=f"aggT{ch}",
                                name=f"aggT{ch}")
                  if ch % 2 == 0:
                      nc.scalar.copy(out=a[:], in_=agg_slice(ch))
                  else:
                      nc.vector.tensor_copy(out=a[:], in_=agg_slice(ch))
                  agg_sb.append(a)

              if debug:
                  for ch in range(NCHAIN):
                      nc.sync.dma_start(
                          out=dbg_d[:, ch * P:(ch + 1) * P].bitcast(f32r),
                          in_=agg_sb[ch][:])
                  nc.sync.dma_start(
                      out=dbg_d[:, NCHAIN * P:(NCHAIN + 1) * P].bitcast(f32r),
                      in_=self_sb[0][:])

          # ---- SAGE layer in transposed layout (agg PSUM banks now free) --
          with tc.tile_pool(name="psage", bufs=8, space="PSUM") as psage:
            h1n_all = acts.tile([P, S2 * P], f32r, tag="h1n_all")

            def sage_group(specs):
                """Stage-major emission of several independent SAGE heads so
                the engines pipeline across them."""
                phs, hs, h2s, psss, nvs, pbcs, nrs = [], [], [], [], [], [], []
                for i, (rs, ra, wa, wb_, bt, tagn, hn) in enumerate(specs):
                    ph = psage.tile([P, P], f32, tag="ps", name=f"ph_{tagn}")
                    nc.tensor.matmul(out=ph[:], lhsT=wa[:], rhs=rs,
                                     start=True, stop=False,
                                     skip_group_check=True)
                    nc.tensor.matmul(out=ph[:], lhsT=wb_[:], rhs=ra,
                                     start=False, stop=True,
                                     skip_group_check=True)
                    phs.append(ph)
                for i, (rs, ra, wa, wb_, bt, tagn, hn) in enumerate(specs):
                    h = scratch.tile([P, P], f32, tag="h", name=f"h_{tagn}")
                    nc.vector.tensor_scalar(out=h[:], in0=phs[i][:],
                                            scalar1=bt[:, :1], scalar2=0.0,
                                            op0=mybir.AluOpType.add,
                                            op1=mybir.AluOpType.max)
                    hs.append(h)
                for i, (rs, ra, wa, wb_, bt, tagn, hn) in enumerate(specs):
                    h2 = scratch.tile([P, P], f32r, tag="h2", name=f"h2_{tagn}")
                    nc.scalar.square(out=h2[:], in_=hs[i][:])
                    h2s.append(h2)
                for i, (rs, ra, wa, wb_, bt, tagn, hn) in enumerate(specs):
                    pss = psage.tile([P, P], f32, tag="ps", name=f"pss_{tagn}")
                    nc.tensor.matmul(out=pss[:1, :], lhsT=ones[:, :1],
                                     rhs=h2s[i][:], start=True, stop=True,
                                     skip_group_check=True)
                    psss.append(pss)
                for i, (rs, ra, wa, wb_, bt, tagn, hn) in enumerate(specs):
                    nv = scratch.tile([P, P], f32r, tag="nv", name=f"nv_{tagn}")
                    nc.scalar.activation(
                        out=nv[:1, :], in_=psss[i][:1, :],
                        func=mybir.ActivationFunctionType.Sqrt,
                        bias=eps[:1, :1])
                    nvs.append(nv)
                for i, (rs, ra, wa, wb_, bt, tagn, hn) in enumerate(specs):
                    pbc = psage.tile([P, P], f32, tag="ps", name=f"pbc_{tagn}")
                    nc.tensor.matmul(out=pbc[:], lhsT=ones[:1, :],
                                     rhs=nvs[i][:1, :], start=True, stop=True,
                                     skip_group_check=True)
                    pbcs.append(pbc)
                for i, (rs, ra, wa, wb_, bt, tagn, hn) in enumerate(specs):
                    nr = scratch.tile([P, P], f32, tag="nr", name=f"nr_{tagn}")
                    nc.vector.reciprocal_approx_fast(out=nr[:], in_=pbcs[i][:])
                    nrs.append(nr)
                outs = []
                for i, (rs, ra, wa, wb_, bt, tagn, hn) in enumerate(specs):
                    if hn is None:
                        hn = acts.tile([D, B_LOC], f32r, tag=tagn,
                                       name=tagn)[:]
                    nc.vector.tensor_tensor(out=hn, in0=hs[i][:], in1=nrs[i][:],
                                            op=mybir.AluOpType.mult)
                    outs.append(hn)
                return outs

            h1n_slice = lambda j: h1n_all[:, j * P:(j + 1) * P]
            specs = [(self_sb[0][:], agg_sb[0][:], w1a, w1b, b1t, "h1t", None)]
            specs += [(self_sb[1 + j][:], agg_sb[1 + j][:], w1a, w1b, b1t,
                       f"h1n{j}", h1n_slice(j)) for j in range(S2)]
            res0 = sage_group(specs)
            h1t = res0[0]

            a3 = acts.tile([D, B_LOC], f32r, tag="a3")
            with nc.allow_low_precision("f32r is 4-byte fp32 bits"):
                nc.vector.reduce_sum(
                    out=a3[:],
                    in_=h1n_all[:].rearrange("p (j r) -> p r j", j=S2),
                    axis=mybir.AxisListType.X,
                )

            zt = sage_group([(h1t, a3[:], w2a, w2b, b2t, "zt", None)])[0]
            nc.sync.dma_start(out=zt_d[:].bitcast(f32r), in_=zt)

    nc.finalize()
    return nc


def kernel(x, W1, b1, W2, b2, nodes, nb2, nb1_self, nb1_nb,
           _trace=False, _core_ids=None, _debug=False):
    x = np.ascontiguousarray(np.asarray(x, dtype=np.float32))
    W1 = np.asarray(W1, dtype=np.float32)
    W2 = np.asarray(W2, dtype=np.float32)
    b1 = np.asarray(b1, dtype=np.float32)
    b2 = np.asarray(b2, dtype=np.float32)
    nodes = np.asarray(nodes)
    nb2 = np.asarray(nb2)
    nb1_self = np.asarray(nb1_self)
    nb1_nb = np.asarray(nb1_nb)

    meta = _prep_indices(nodes, nb2, nb1_self, nb1_nb)
    nc = _build_program(meta, debug=_debug)

    try:
        import ml_dtypes
        bf16_np = ml_dtypes.bfloat16
    except ImportError:
        bf16_np = None

    def to_bf16(a):
        if bf16_np is not None:
            return np.ascontiguousarray(a.astype(bf16_np))
        b = a.astype(np.float32).view(np.uint32)
        b = ((b + 0x8000 + ((b >> 16) & 1)) >> 16).astype(np.uint16)
        return np.ascontiguousarray(b)

    # host-prescaled weights: the 1/25 and 1/10 means fold into W*b
    w1a = np.ascontiguousarray(W1[:D])
    w1b = np.ascontiguousarray(W1[D:] / S1)
    w2a = np.ascontiguousarray(W2[:D])
    w2b = np.ascontiguousarray(W2[D:] / S2)
    ident = np.eye(P, dtype=np.float32)
    x16 = to_bf16(x)
    W_max = meta["W_max"]
    Cmax = max(meta["Cb"])
    iota1 = np.tile(np.arange(W_max, dtype=np.float32), Cmax)
    iota = np.broadcast_to(iota1, (P, Cmax * W_max))
    iota16 = to_bf16(iota)

    in_maps = []
    for c in range(NCORES):
        sl = slice(c * B_LOC, (c + 1) * B_LOC)
        idx32 = np.empty((P, NCHAIN), np.int32)
        idx32[:, 0] = nodes[sl]
        idx32[:, 1:] = nb2[sl]
        in_maps.append({
            "x": x16,
            "w1a": w1a, "w1b": w1b, "w2a": w2a, "w2b": w2b,
            "b1v": b1.reshape(D, 1), "b2v": b2.reshape(D, 1),
            "ident": ident, "iota": iota16,
            "onesm": np.ones((P, P), np.float32),
            "idx16": meta["idx16_cores"][c],
            "grp": to_bf16(meta["grp_tile_cores"][c]),
            "idx32": idx32,
        })

    from concourse.bass_utils import run_bass_kernel_spmd

    core_ids = _core_ids if _core_ids is not None else list(range(NCORES))
    res = run_bass_kernel_spmd(nc, in_maps[:len(core_ids)], core_ids=core_ids,
                               trace=_trace)
    z = np.concatenate([res.results[c]["zt"].T for c in range(len(core_ids))],
                       axis=0)
    kernel.last_exec_time_ns = res.exec_time_ns
    kernel.last_results = res
    return z
